# revision 1
# baseline (speedup 1.0000x reference)
"""Trainium2 Bass kernel for DescriptorMatchLoss (retrieval_knn).

Reference computation (per batch-pair grid [B,B]):
    d2[i,j,n,m] = ||denorm(pts_src[i,n]) - denorm(pts_dst[i,j,m])||^2
    mask        = d2 <= RADIUS^2
    cos[i,j,n,m] = <fhat[j,n], fhat[i,m]>   (fhat = row-normalized features)
    loss = sum(mask * (1 - cos)) / max(sum(mask), 1)

Device strategy (8 cores, 2 (i,j) pairs per core):
  * z = 64 - d2 tile [128n, mw] via one K=14 bf16 matmul: coordinates are
    split into (hi, lo) bf16 pairs so every product is exact in fp32 PSUM
    (full PE rate; native fp32 matmul is 4x slower).
  * Mask tiles in SBUF bf16, produced alternately by the ACT engine
    (sign(z) in {-1,0,+1}, fused count accumulation) and the DVE
    (z >= 0 in {1,0}) so PSUM slots recycle fast enough to keep PE fed.
  * PE contracts G[m,d] = sum_n mask[n,m] * fhat[j][n,d] (mask stationary,
    K=128 per n-tile, accumulated over 16 n-tiles in PSUM).
  * DVE multiply+reduce: ext = sum_{m,d} G[m,d]*fhat[i][m,d]
    = sum_{n,m} mask[n,m]*cos[n,m].
  * Host: exact affine correction for the +-1 tiles (sum of cos over a
    full n-range x m-chunk factorizes into dots of feature column sums).

kernel(**inputs) takes FULL inputs, shards pairs across 8 cores, returns the
scalar loss (fp32).
"""

import sys

for _p in ("/opt/pypackages", "/opt/trn_rl_repo"):
    if _p not in sys.path:
        sys.path.insert(0, _p)

import numpy as np
import ml_dtypes

BF16 = ml_dtypes.bfloat16

# Problem constants (hardcoded per contract).
B, N, D = 4, 2048, 256
HEIGHT, WIDTH = 480, 640
RADIUS2 = 64.0
N_CORES = 8
PAIRS_PER_CORE = (B * B) // N_CORES  # 2

P = 128          # partitions
NT = N // P      # 16 n-tiles of 128
DC = D // P      # feature-dim chunks (2)
KGEO = 14        # geometry contraction rows

# Tunables (kernel structure); _host_prep must agree on MW/engine split.
MW = 512         # m-tile width
MT = N // MW     # m-tiles per pair
MC = MW // P     # m-chunks of 128 per m-tile
D2_BUFS = 6
G_BUFS = 2
MASK_BUFS = 7
PIPE = True      # software-pipeline G one step behind d2/sign
REPS = 1         # repeat compute loop (timing only)
USE_TTR = False  # fused multiply+reduce extraction (walrus rejects)
CARRIER = False  # tiny PE matmul absorbing the g-slot WAR wait (the hoisted
                 # eventsem from _split_multi_waits is cheaper on HW)
EXT_PATH = "dve"  # "dve": DVE TT+reduce from PSUM; "pool": ACT copy ->
                  # GpSimd multiply -> DVE reduce (spreads extraction load)
FJ_FP8 = True    # fp8e4m3 fj + masks, G matmul in DoubleRow mode (2x fewer
                 # PE contraction steps; loss impact ~2e-6 rel, host-corrected
                 # exactly via fp8 column sums)


# Per-step engine pattern for mask production, chosen to balance engine
# load (DVE also runs the extraction): 5 ACT steps, 3 DVE steps, with the
# pipeline's step-pairs mixed (ACT, DVE) where possible.
MASK_PATTERN = ["act", "dve", "act", "dve", "act", "dve", "act", "act"]


def _mask_engine(pair, mt, nt=None):
    """Which engine produces the mask for (pair, mt): 'act' (+-1 sign
    convention) or 'dve' ({0,1} convention); uniform per step so the
    host-side affine correction stays exact."""
    return MASK_PATTERN[(pair * MT + mt) % len(MASK_PATTERN)]


_CACHE = {}
LAST = None  # BassKernelResults of the most recent run (for test harness)


def _build_bass(reps=None, mode="full", split_waits=True):
    import concourse.bass as bass
    import concourse.mybir as mybir
    import concourse.tile as tile

    if reps is None:
        reps = REPS

    nc = bass.Bass(trn_type="TRN2", target_bir_lowering=False, debug=False)
    f32 = mybir.dt.float32
    bf16 = mybir.dt.bfloat16

    mdt = mybir.dt.float8e4 if FJ_FP8 else bf16  # fj + mask dtype
    fj_d = nc.dram_tensor("fj", [PAIRS_PER_CORE, N, D], mdt, kind="ExternalInput")
    # fiT: host-transposed normalized features of the i-batches, [pairs, D, N]
    fi_d = nc.dram_tensor("fiT", [PAIRS_PER_CORE, D, N], bf16, kind="ExternalInput")
    geoL_d = nc.dram_tensor(
        "geoL", [PAIRS_PER_CORE, KGEO, N], bf16, kind="ExternalInput"
    )
    geoR_d = nc.dram_tensor(
        "geoR", [PAIRS_PER_CORE, KGEO, N], bf16, kind="ExternalInput"
    )
    # out[:, 0:PAIRS*MT*NT]      : per-(pair, m-tile, n-tile) mask sums
    # out[:, PAIRS*MT*NT:+32]    : per-(pair, m-chunk) mask*cos sums
    n_sgn = PAIRS_PER_CORE * MT * NT
    n_ext = PAIRS_PER_CORE * MT * DC
    out_d = nc.dram_tensor("out", [P, n_sgn + n_ext], f32, kind="ExternalOutput")

    steps = [(pair, mt) for pair in range(PAIRS_PER_CORE) for mt in range(MT)]

    with tile.TileContext(nc) as tc:
        with (
            tc.tile_pool(name="feat", bufs=1) as feat_pool,
            tc.tile_pool(name="geo", bufs=1) as geo_pool,
            tc.tile_pool(name="acc", bufs=1) as acc_pool,
            tc.tile_pool(name="mask", bufs=MASK_BUFS) as mask_pool,
            tc.tile_pool(name="scratch", bufs=2) as scratch_pool,
            tc.tile_pool(name="psum_d2", bufs=D2_BUFS, space="PSUM") as d2_pool,
            tc.tile_pool(name="psum_g", bufs=G_BUFS, space="PSUM") as g_pool,
        ):
            fj_sb = feat_pool.tile([P, PAIRS_PER_CORE, NT, D], mdt)
            fi_sb = feat_pool.tile([P, PAIRS_PER_CORE, DC, N], bf16)
            # Geometry replicated at partition offsets 0/32/64/96 so four
            # K=14 d2 matmuls can run concurrently in the four PE row groups.
            geoL_sb = geo_pool.tile([P, PAIRS_PER_CORE, N], bf16)
            geoR_sb = geo_pool.tile([P, PAIRS_PER_CORE, N], bf16)
            sgn_acc = acc_pool.tile([P, n_sgn], f32)
            ext_acc = acc_pool.tile([P, n_ext], f32)
            if mode != "full":
                nc.vector.memset(sgn_acc[:], 0.0)
                nc.vector.memset(ext_acc[:], 0.0)

            for rg in range(4):
                nc.sync.dma_start(
                    out=geoL_sb[32 * rg : 32 * rg + KGEO, :, :],
                    in_=geoL_d[:].rearrange("q k n -> k q n"),
                )
                nc.sync.dma_start(
                    out=geoR_sb[32 * rg : 32 * rg + KGEO, :, :],
                    in_=geoR_d[:].rearrange("q k n -> k q n"),
                )
            nc.sync.dma_start(
                out=fj_sb[:], in_=fj_d[:].rearrange("q (t p) d -> p q t d", p=P)
            )
            nc.sync.dma_start(
                out=fi_sb[:], in_=fi_d[:].rearrange("q (c p) n -> p q c n", p=P)
            )

            # DMA-tick absorbers: each engine "observes" the input-DMA
            # completion once via a cheap op, so later instructions inherit
            # the tick through the vector clock and mostly carry a single
            # cross-engine wait.
            dummy_ps = g_pool.tile([1, 8], f32, tag="g")
            dummy_sb = scratch_pool.tile([1, 8], f32, tag="dmy")
            nc.tensor.matmul(
                dummy_ps[:, 0:8], geoL_sb[0:KGEO, 0, 0:1], geoL_sb[0:KGEO, 0, 0:8],
                start=True, stop=True,
            )
            nc.tensor.matmul(
                dummy_ps[:, 0:8], geoR_sb[0:KGEO, 0, 0:1], geoR_sb[0:KGEO, 0, 0:8],
                start=True, stop=True,
            )
            nc.tensor.matmul(
                dummy_ps[:, 0:8], fj_sb[:, 0, 0, 0:1], fj_sb[:, 0, 0, 0:8],
                start=True, stop=True,
            )
            nc.vector.tensor_copy(dummy_sb[0:1, 0:1], fi_sb[0:1, 0, 0, 0:1])
            nc.scalar.copy(dummy_sb[0:1, 1:2], dummy_sb[0:1, 0:1])

            def emit_mask_op(pair, mt, nt, d2_ps, mask_t):
                col = (pair * MT + mt) * NT + nt
                eng = _mask_engine(pair, mt, nt)
                if eng == "act":
                    nc.scalar.activation(
                        mask_t[:, nt, :],
                        d2_ps[:],
                        mybir.ActivationFunctionType.Sign,
                        accum_out=sgn_acc[:, col : col + 1],
                    )
                else:
                    nc.vector.tensor_scalar(
                        out=mask_t[:, nt, :],
                        in0=d2_ps[:],
                        scalar1=0.0,
                        scalar2=0.0,
                        op0=mybir.AluOpType.is_ge,
                        op1=mybir.AluOpType.add,
                        accum_out=sgn_acc[:, col : col + 1],
                    )

            def emit_d2_quad(pair, mt, nt0, mask_t):
                """Four K=14 d2 matmuls packed into the four PE row groups
                (concurrent execution; weight loads overlap other groups'
                in-flight matmuls), then their mask ops."""
                tiles = []
                for k in range(4):
                    nt = nt0 + k
                    d2_ps = d2_pool.tile([P, MW], f32, tag="d2")
                    nc.tensor.matmul(
                        d2_ps[:],
                        geoL_sb[32 * k : 32 * k + KGEO, pair,
                                nt * P : (nt + 1) * P],
                        geoR_sb[32 * k : 32 * k + KGEO, pair,
                                mt * MW : (mt + 1) * MW],
                        start=True,
                        stop=True,
                        tile_position=(32 * k, 0),
                    )
                    tiles.append((nt, d2_ps))
                for nt, d2_ps in tiles:
                    emit_mask_op(pair, mt, nt, d2_ps, mask_t)

            def emit_d2_sign(pair, mt):
                mask_t = mask_pool.tile([P, NT, MW], mdt, tag="mask")
                for q in range(NT // 4):
                    emit_d2_quad(pair, mt, 4 * q, mask_t)
                return mask_t

            def emit_d2_sign_pair(sa, sb):
                """Interleave two steps' d2+mask production so the ACT-masked
                and DVE-masked streams run concurrently."""
                ma = mask_pool.tile([P, NT, MW], mdt, tag="mask")
                mb = mask_pool.tile([P, NT, MW], mdt, tag="mask")
                for q in range(NT // 4):
                    emit_d2_quad(sa[0], sa[1], 4 * q, ma)
                    emit_d2_quad(sb[0], sb[1], 4 * q, mb)
                return ma, mb

            def emit_g_half(pair, mt, mask_t, dc, half, g_ps):
                # G^T[d, m] = sum_n fhat_j[n, d] * mask[n, m]: stationary is
                # the fhat_j (n x d-chunk) tile, moving is the full [128, MW]
                # mask tile, so each matmul streams MW columns and the weight
                # load hides behind the previous matmul.
                if half == 0 and CARRIER:
                    # Carrier: absorb the WAR wait on this PSUM slot (its
                    # previous reader was the DVE extraction) into a tiny
                    # matmul so the real group's first matmul only waits
                    # on the mask writes.
                    nc.tensor.matmul(
                        g_ps[0:1, 0:1],
                        geoL_sb[0:KGEO, pair, 0:1],
                        geoR_sb[0:KGEO, pair, 0:1],
                        start=True,
                        stop=True,
                    )
                if FJ_FP8:
                    # DoubleRow: each matmul contracts TWO n-tiles (fp8 pairs
                    # interleaved along the middle AP dim).
                    nps = NT // 2  # 8 nt-pairs
                    prs = range(nps // 2) if half == 0 else range(nps // 2, nps)
                    for t in prs:
                        nc.tensor.matmul(
                            g_ps[:],
                            fj_sb[:, pair, 2 * t : 2 * t + 2,
                                  dc * P : (dc + 1) * P],
                            mask_t[:, 2 * t : 2 * t + 2, :],
                            start=(t == 0),
                            stop=(t == nps - 1),
                            perf_mode=mybir.MatmulPerfMode.DoubleRow,
                        )
                else:
                    nts = range(NT // 2) if half == 0 else range(NT // 2, NT)
                    for nt in nts:
                        nc.tensor.matmul(
                            g_ps[:],
                            fj_sb[:, pair, nt, dc * P : (dc + 1) * P],
                            mask_t[:, nt, :],
                            start=(nt == 0),
                            stop=(nt == NT - 1),
                        )
                if half == 1:
                    ecol = (pair * MT + mt) * DC + dc
                    scr = scratch_pool.tile([P, MW], f32, tag="scr")
                    if USE_TTR:
                        nc.vector.tensor_tensor_reduce(
                            out=scr[:],
                            in0=g_ps[:],
                            in1=fi_sb[:, pair, dc, mt * MW : (mt + 1) * MW],
                            scale=1.0,
                            scalar=0.0,
                            op0=mybir.AluOpType.mult,
                            op1=mybir.AluOpType.add,
                            accum_out=ext_acc[:, ecol : ecol + 1],
                        )
                    elif EXT_PATH == "pool":
                        g_sb = scratch_pool.tile([P, MW], f32, tag="gsb")
                        nc.scalar.copy(g_sb[:], g_ps[:])
                        nc.gpsimd.tensor_tensor(
                            out=scr[:],
                            in0=g_sb[:],
                            in1=fi_sb[:, pair, dc, mt * MW : (mt + 1) * MW],
                            op=mybir.AluOpType.mult,
                        )
                        nc.vector.tensor_reduce(
                            out=ext_acc[:, ecol : ecol + 1],
                            in_=scr[:],
                            axis=mybir.AxisListType.X,
                            op=mybir.AluOpType.add,
                        )
                    else:
                        nc.vector.tensor_tensor(
                            out=scr[:],
                            in0=g_ps[:],
                            in1=fi_sb[:, pair, dc, mt * MW : (mt + 1) * MW],
                            op=mybir.AluOpType.mult,
                        )
                        nc.vector.tensor_reduce(
                            out=ext_acc[:, ecol : ecol + 1],
                            in_=scr[:],
                            axis=mybir.AxisListType.X,
                            op=mybir.AluOpType.add,
                        )

            def g_units(pair, mt, mask_t):
                units = []
                for dc in range(DC):
                    g_ps = g_pool.tile([P, MW], f32, tag="g")
                    for half in range(2):
                        units.append(
                            (lambda p=pair, m=mt, k=mask_t, d=dc, h=half,
                             g=g_ps: emit_g_half(p, m, k, d, h, g))
                        )
                return units

            def emit_g(pair, mt, mask_t):
                for u in g_units(pair, mt, mask_t):
                    u()

            def emit_body(mode):
                if mode == "d2sign":
                    for s in steps:
                        emit_d2_sign(*s)
                elif mode == "d2only":
                    for pair, mt in steps:
                        for q in range(NT // 4):
                            for k in range(4):
                                nt = 4 * q + k
                                d2_ps = d2_pool.tile([P, MW], f32, tag="d2")
                                nc.tensor.matmul(
                                    d2_ps[:],
                                    geoL_sb[32 * k : 32 * k + KGEO, pair,
                                            nt * P : (nt + 1) * P],
                                    geoR_sb[32 * k : 32 * k + KGEO, pair,
                                            mt * MW : (mt + 1) * MW],
                                    start=True,
                                    stop=True,
                                    tile_position=(32 * k, 0),
                                )
                elif mode == "gonly":
                    mask_const = mask_pool.tile([P, NT, MW], mdt, tag="mask")
                    nc.vector.memset(mask_const[:], 1.0)
                    for pair, mt in steps:
                        emit_g(pair, mt, mask_const)
                elif PIPE:
                    # Software pipeline: phase k's d2-quads+masks interleave
                    # with phase k-1's G units so the PE's in-order queue
                    # always has ready G work while masks drain d2 slots.
                    prev_units = None
                    for k in range(0, len(steps), 2):
                        sa, sb = steps[k], steps[k + 1]
                        ma = mask_pool.tile([P, NT, MW], mdt, tag="mask")
                        mb = mask_pool.tile([P, NT, MW], mdt, tag="mask")
                        quads = []
                        for q in range(NT // 4):
                            quads.append(
                                lambda s=sa, m=ma, q0=4 * q:
                                    emit_d2_quad(s[0], s[1], q0, m)
                            )
                            quads.append(
                                lambda s=sb, m=mb, q0=4 * q:
                                    emit_d2_quad(s[0], s[1], q0, m)
                            )
                        for idx, qu in enumerate(quads):
                            # Quad first: the d2 quads feed the mask engines
                            # as early as possible (measured better than
                            # G-unit-first, which starves mask production).
                            qu()
                            if prev_units is not None:
                                prev_units[idx]()
                        prev_units = (
                            g_units(sa[0], sa[1], ma) + g_units(sb[0], sb[1], mb)
                        )
                    for u in prev_units:
                        u()
                else:
                    for s in steps:
                        m = emit_d2_sign(*s)
                        emit_g(s[0], s[1], m)

            if reps == 1:
                emit_body(mode)
            else:
                with tc.For_i(0, reps, 1):
                    emit_body(mode)

            nc.sync.dma_start(out=out_d[:, 0:n_sgn], in_=sgn_acc[:])
            nc.sync.dma_start(out=out_d[:, n_sgn : n_sgn + n_ext], in_=ext_acc[:])

    if split_waits:
        _split_multi_waits(nc)
    return nc


def _split_multi_waits(nc):
    """Walrus rejects >1 sync-wait on compute/DMA instruction encodings.

    Hoist all but one wait of any multi-wait instruction onto standalone
    InstEventSemaphore instructions inserted immediately before it on the
    same engine queue (semantically identical: every wait must pass before
    the instruction dispatches either way).
    """
    import concourse.mybir as mybir

    n_split = 0
    for bb in nc.main_func.blocks:
        new_list = []
        for inst in bb.instructions:
            si = inst.sync_info
            if (
                si is not None
                and si.on_wait
                and len(si.on_wait) > 1
                and not isinstance(inst, mybir.InstEventSemaphore)
            ):
                waits = list(si.on_wait)
                for k, w in enumerate(waits[:-1]):
                    n_split += 1
                    new_list.append(
                        mybir.InstEventSemaphore(
                            name=f"{inst.name}-hw{k}",
                            engine=inst.engine,
                            ins=[],
                            outs=[],
                            sync_info=mybir.SyncInfo(on_wait=[w], on_update=[]),
                        )
                    )
                inst.sync_info = mybir.SyncInfo(
                    on_wait=[waits[-1]], on_update=list(si.on_update or [])
                )
            new_list.append(inst)
        bb.instructions[:] = new_list
    return n_split


def _get_bass():
    if "nc" not in _CACHE:
        _CACHE["nc"] = _build_bass()
    return _CACHE["nc"]


def _split2(x):
    """fp64 -> (hi, lo) bf16 such that hi+lo ~ x to ~17 mantissa bits."""
    hi = x.astype(BF16)
    lo = (x - hi.astype(np.float64)).astype(BF16)
    return hi, lo


def _split3(x):
    hi = x.astype(BF16)
    r = x - hi.astype(np.float64)
    mid = r.astype(BF16)
    lo = (r - mid.astype(np.float64)).astype(BF16)
    return hi, mid, lo


def _host_prep(features, pts_src, pts_dst, height, width):
    """Build per-core device inputs + exact host-side correction terms."""
    height = int(height)
    width = int(width)
    scale32 = np.array(
        [(width - 1) * 0.5, (height - 1) * 0.5], dtype=np.float32
    )

    # Match the reference's fp32 denormalization rounding, then center (the
    # centering offset equals `scale`, so centered coords = denorm - scale).
    ps32 = (pts_src.astype(np.float32) + np.float32(1.0)) * scale32  # [B,N,2]
    pd32 = (pts_dst.astype(np.float32) + np.float32(1.0)) * scale32  # [B,B,N,2]
    psc = ps32.astype(np.float64) - scale32.astype(np.float64)
    pdc = pd32.astype(np.float64) - scale32.astype(np.float64)

    phx, plx = _split2(psc[..., 0])
    phy, ply = _split2(psc[..., 1])
    qhx, qlx = _split2(pdc[..., 0])
    qhy, qly = _split2(pdc[..., 1])

    # The PSUM result is z = 64 - d2 = 2 p.q + (64 - s_src) - s_dst, so the
    # mask is sign(z) / (z >= 0) with no activation bias needed.  s terms are
    # computed from the *split* values so the only error is the residual.
    sh, sm, sl = _split3(
        RADIUS2
        - (
            (phx.astype(np.float64) + plx.astype(np.float64)) ** 2
            + (phy.astype(np.float64) + ply.astype(np.float64)) ** 2
        )
    )  # [B,N]
    tq = (
        (qhx.astype(np.float64) + qlx.astype(np.float64)) ** 2
        + (qhy.astype(np.float64) + qly.astype(np.float64)) ** 2
    )
    th, tm, tl = _split3(tq)  # [B,B,N]

    ones_bn = np.ones((B, N), dtype=BF16)
    ones_bbn = np.ones((B, B, N), dtype=BF16)
    neg_ones_bn = -ones_bn

    p2hx = (2.0 * phx.astype(np.float64)).astype(BF16)
    p2lx = (2.0 * plx.astype(np.float64)).astype(BF16)
    p2hy = (2.0 * phy.astype(np.float64)).astype(BF16)
    p2ly = (2.0 * ply.astype(np.float64)).astype(BF16)
    geoL_all = np.stack(
        [p2hx, p2hx, p2lx, p2lx, p2hy, p2hy, p2ly, p2ly,
         sh, sm, sl, neg_ones_bn, neg_ones_bn, neg_ones_bn],
        axis=1,
    )  # [B, 14, N]
    geoR_all = np.stack(
        [qhx, qlx, qhx, qlx, qhy, qly, qhy, qly,
         ones_bbn, ones_bbn, ones_bbn, th, tm, tl],
        axis=2,
    )  # [B, B, 14, N]

    # Normalized features, rounded to bf16 (the dtype used on device).
    f64 = features.astype(np.float64)
    norms = np.sqrt((f64 * f64).sum(-1, keepdims=True))
    fhat = (f64 / norms).astype(BF16)  # [B, N, D]
    if FJ_FP8:
        fhat_j = fhat.astype(ml_dtypes.float8_e4m3)  # device fj operand
    else:
        fhat_j = fhat

    # Per-m-chunk column sums for the +-1 correction (exact, fp64 over the
    # same quantized values the device uses: fj-side dtype for `fsum`,
    # bf16 fiT for `fsum_chunk`).
    fsum_chunk = fhat.astype(np.float64).reshape(B, NT, P, D).sum(axis=2)
    fsum = fhat_j.astype(np.float64).sum(axis=1)  # [B, D]

    in_maps = []
    pair_idx = []  # per core: list of (i, j)
    for c in range(N_CORES):
        pairs = [2 * c, 2 * c + 1]
        ii = [p // B for p in pairs]
        jj = [p % B for p in pairs]
        in_maps.append(
            {
                "fj": np.ascontiguousarray(fhat_j[jj]),
                "fiT": np.ascontiguousarray(fhat[ii].transpose(0, 2, 1)),
                "geoL": np.ascontiguousarray(geoL_all[ii]),
                "geoR": np.ascontiguousarray(
                    np.stack([geoR_all[i_, j_] for i_, j_ in zip(ii, jj)])
                ),
            }
        )
        pair_idx.append(list(zip(ii, jj)))
    return in_maps, pair_idx, fsum, fsum_chunk


def _combine(results, pair_idx, fsum, fsum_chunk, cores=None):
    """Host-side exact combination of per-core partial sums."""
    if cores is None:
        cores = range(len(results))
    n_sgn = PAIRS_PER_CORE * MT * NT
    a_total = 0.0
    b_total = 0.0
    for c in cores:
        out = results[c]["out"].astype(np.float64)
        sgn_p = out[:, 0:n_sgn]                    # per-partition accum values
        ext = out[:, n_sgn:].sum(axis=0)           # per (pair, mt, dc) col
        for p, (i_, j_) in enumerate(pair_idx[c]):
            for mt in range(MT):
                for nt in range(NT):
                    eng = _mask_engine(p, mt, nt)
                    col = sgn_p[:, (p * MT + mt) * NT + nt]
                    if eng == "act":
                        # sum of +-1 per partition over MW elements
                        a_total += 0.5 * (float(col.sum()) + P * MW)
                    else:
                        a_total += float(col.sum())  # {0,1} masks
            for mt in range(MT):
                eng0 = _mask_engine(p, mt, 0)
                # m-tile column sums of fhat_i over this tile's m range
                fs_mt = fsum_chunk[i_, mt * MC : (mt + 1) * MC].sum(axis=0)
                for dc in range(DC):
                    e = float(ext[(p * MT + mt) * DC + dc])
                    if eng0 == "act":
                        # +-1 convention
                        dsl = slice(dc * P, (dc + 1) * P)
                        corr = float(np.dot(fsum[j_][dsl], fs_mt[dsl]))
                        b_total += 0.5 * (e + corr)
                    else:
                        b_total += e
    return a_total, b_total


def kernel(features, pts_src, pts_dst, invis_idx, height, width):
    global LAST
    del invis_idx  # unused by the reference computation

    features = np.asarray(features)
    pts_src = np.asarray(pts_src)
    pts_dst = np.asarray(pts_dst)

    in_maps, pair_idx, fsum, fsum_chunk = _host_prep(
        features, pts_src, pts_dst, height, width
    )

    from concourse.bass_utils import run_bass_kernel_spmd

    nc = _get_bass()
    LAST = run_bass_kernel_spmd(nc, in_maps, core_ids=list(range(N_CORES)))

    a_total, b_total = _combine(LAST.results, pair_idx, fsum, fsum_chunk)
    loss = (a_total - b_total) / max(a_total, 1.0)
    return np.float32(loss)



# revision 14
# speedup vs baseline: 1.3569x; 1.3569x over previous
"""Trainium2 Bass kernel for DescriptorMatchLoss (retrieval_knn).

Reference computation (per batch-pair grid [B,B]):
    d2[i,j,n,m] = ||denorm(pts_src[i,n]) - denorm(pts_dst[i,j,m])||^2
    mask        = d2 <= RADIUS^2
    cos[i,j,n,m] = <fhat[j,n], fhat[i,m]>   (fhat = row-normalized features)
    loss = sum(mask * (1 - cos)) / max(sum(mask), 1)

Key structure (v2, x-sorted windowed):
  * Host sorts src points and dst points by x-coordinate per pair (the loss
    is permutation invariant).  A 512-wide m-tile of sorted dst then only
    interacts with the contiguous window of src n-tiles whose x range
    overlaps within +-RADIUS: ~5-6 of 16 n-tiles.  Coverage is exact by
    construction (searchsorted with margin); work drops ~3x.  All 8 cores
    share one SPMD program, so the per-step window is the union over cores
    (near-identical because x is ~uniform in every batch).
  * z = 64 - d2 tiles [128n, 512m] via K=14 bf16 matmuls (hi/lo split
    coordinates, exact to ~2^-17); geometry replicated at partition
    offsets 0/32/64/96 so four matmuls pack into the four PE row groups.
  * Masks in fp8 produced by ACT (Sign -> {-1,+1}) or DVE (is_ge -> {0,1})
    on [128, 2, 512] double-tiles (two PSUM banks per op), with fused
    count accumulation.  Engine uniform per (pair, m-tile) step so the
    host-side affine correction for the +-1 convention stays exact.
  * PE contracts G[d,m] = sum_n mask[n,m]*fhat_j[n,d] over the window
    tiles (fp8 DoubleRow, 2 n-tiles per matmul).
  * DVE fused multiply+reduce (scalar_tensor_tensor with accum_out):
    ext = sum_{m,d} G[d,m]*fhat_i[d,m] in one pass.
  * Inputs land via three parallel DMA queues (SP / ACT / Pool SWDGE),
    dense [128, x] layouts prepacked on host.

kernel(**inputs) takes FULL inputs, shards the 16 (i,j) pairs across 8
cores (2 pairs/core), returns the scalar loss (fp32).
"""

import sys

for _p in ("/opt/pypackages", "/opt/trn_rl_repo"):
    if _p not in sys.path:
        sys.path.insert(0, _p)

import numpy as np
import ml_dtypes

BF16 = ml_dtypes.bfloat16
FP8 = ml_dtypes.float8_e4m3

# Problem constants (hardcoded per contract).
B, N, D = 4, 2048, 256
HEIGHT, WIDTH = 480, 640
RADIUS = 8.0
RADIUS2 = 64.0
N_CORES = 8
PAIRS_PER_CORE = (B * B) // N_CORES  # 2

P = 128          # partitions
NT = N // P      # 16 n-tiles of 128
DC = D // P      # feature-dim chunks (2)
KGEO = 14        # geometry contraction rows

MW = 512         # m-tile width
MT = N // MW     # m-tiles per pair (4)
N_STEPS = PAIRS_PER_CORE * MT
MARGIN = RADIUS + 0.01  # x-window margin (fp slack)

# Engine per (pair, mt) step: ACT(+-1 Sign) or DVE({0,1} is_ge).  DVE also
# runs the fused extraction, so ACT takes the larger share of mask steps.
STEP_PATTERN = ["act", "dve", "act", "act", "act", "act", "dve", "act"]

MASK_BUFS = 4
D2_GROUP = 3     # window tiles per mask op (PSUM banks per d2 tile)
D2_BUFS = 2      # [128, D2_GROUP, 512] tiles in flight
G_BUFS = 2       # two dc tiles in flight per step
PIPE = True


def _step_engine(pair, mt):
    return STEP_PATTERN[(pair * MT + mt) % len(STEP_PATTERN)]


def _nops(w):
    return (w + D2_GROUP - 1) // D2_GROUP


_CACHE = {}
LAST = None  # BassKernelResults of the most recent run (for test harness)


def _build_bass(windows, reps=1, split_waits=True):
    """windows: tuple over steps (pair-major, mt-minor) of (lo_nt, w)."""
    import concourse.bass as bass
    import concourse.mybir as mybir
    import concourse.tile as tile

    nc = bass.Bass(trn_type="TRN2", target_bir_lowering=False, debug=False)
    f32 = mybir.dt.float32
    bf16 = mybir.dt.bfloat16
    fp8 = mybir.dt.float8e4

    steps = [(pair, mt) for pair in range(PAIRS_PER_CORE) for mt in range(MT)]
    win = {s: windows[k] for k, s in enumerate(steps)}
    wmax = max(w for (_, w) in windows)

    # Accumulator columns: one per mask op; mask ops per step = ceil(w/D2_GROUP).
    nops = {s: _nops(win[s][1]) for s in steps}
    sgn_col = {}
    c = 0
    for s in steps:
        sgn_col[s] = c
        c += nops[s]
    n_sgn = max(c, 1)
    n_ext = N_STEPS * DC

    # geo: host-prepacked dense [128, pairs, N] bf16 (rows replicated at
    # partition offsets 0/32/64/96; rows 14-31 of each group are zero).
    geoL_d = nc.dram_tensor("geoL", [P, PAIRS_PER_CORE, N], bf16, kind="ExternalInput")
    geoR_d = nc.dram_tensor("geoR", [P, PAIRS_PER_CORE, N], bf16, kind="ExternalInput")
    # fj: [128, pairs, NT, D] fp8 (partition = n within tile)
    fj_d = nc.dram_tensor("fj", [P, PAIRS_PER_CORE, NT, D], fp8, kind="ExternalInput")
    # fiT: [128, pairs, DC, N] fp8 (partition = d within chunk)
    fi_d = nc.dram_tensor("fiT", [P, PAIRS_PER_CORE, DC, N], fp8, kind="ExternalInput")
    out_d = nc.dram_tensor("out", [P, n_sgn + n_ext], f32, kind="ExternalOutput")

    with tile.TileContext(nc) as tc:
        with (
            tc.tile_pool(name="feat", bufs=1) as feat_pool,
            tc.tile_pool(name="geo", bufs=1) as geo_pool,
            tc.tile_pool(name="acc", bufs=1) as acc_pool,
            tc.tile_pool(name="mask", bufs=MASK_BUFS) as mask_pool,
            tc.tile_pool(name="scratch", bufs=2) as scratch_pool,
            tc.tile_pool(name="psum_d2", bufs=D2_BUFS, space="PSUM") as d2_pool,
            tc.tile_pool(name="psum_g", bufs=G_BUFS, space="PSUM") as g_pool,
        ):
            geoL_sb = geo_pool.tile([P, PAIRS_PER_CORE, N], bf16)
            geoR_sb = geo_pool.tile([P, PAIRS_PER_CORE, N], bf16)
            fj_sb = feat_pool.tile([P, PAIRS_PER_CORE, NT, D], fp8)
            fi_sb = feat_pool.tile([P, PAIRS_PER_CORE, DC, N], fp8)
            sgn_acc = acc_pool.tile([P, n_sgn], f32)
            ext_acc = acc_pool.tile([P, n_ext], f32)

            # Input DMAs on three parallel queues.  SP: geoL; ACT: geoR
            # (ACT is idle until geo arrives anyway); Pool SWDGE: fj, fi.
            # geo is split by pair so pair-0 compute starts ~1.6us earlier.
            for q in range(PAIRS_PER_CORE):
                nc.sync.dma_start(out=geoL_sb[:, q, :], in_=geoL_d[:, q, :])
                nc.scalar.dma_start(out=geoR_sb[:, q, :], in_=geoR_d[:, q, :])
            nc.gpsimd.dma_start(out=fj_sb[:], in_=fj_d[:])
            nc.gpsimd.dma_start(out=fi_sb[:], in_=fi_d[:])

            # Preload the Sign activation table off the critical path: a
            # tiny Sign op with no input dependencies right at t=0.
            warm = acc_pool.tile([P, 8], f32)
            nc.scalar.activation(
                warm[:, 0:8], warm[:, 0:8], mybir.ActivationFunctionType.Sign
            )

            # DMA-tick absorbers: each engine observes input-DMA completion
            # once via a cheap op so later instructions inherit the tick.
            # The fj/fi absorbers are emitted lazily just before first use so
            # the PE/DVE queues don't stall on the (later) feature DMAs.
            dummy_ps = g_pool.tile([1, 8], f32, tag="g")
            dummy_sb = scratch_pool.tile([1, 8], f32, tag="scr")
            nc.tensor.matmul(
                dummy_ps[:, 0:8], geoL_sb[0:KGEO, 0, 0:1], geoL_sb[0:KGEO, 0, 0:8],
                start=True, stop=True,
            )
            nc.tensor.matmul(
                dummy_ps[:, 0:8], geoR_sb[0:KGEO, 0, 0:1], geoR_sb[0:KGEO, 0, 0:8],
                start=True, stop=True,
            )
            feat_tick = [False]

            def absorb_feat_tick():
                if feat_tick[0]:
                    return
                feat_tick[0] = True
                nc.tensor.matmul(
                    dummy_ps[:, 0:8], fj_sb[:, 0, 0, 0:1], fj_sb[:, 0, 0, 0:8],
                    start=True, stop=True,
                )
                nc.vector.tensor_copy(dummy_sb[0:1, 0:1], fi_sb[0:1, 0, 0, 0:1])

            quad_ctr = [0]

            def emit_d2_op(pair, mt, k, mask_t):
                """One mask op: d2 matmuls for up to D2_GROUP window tiles
                into a multi-bank PSUM tile, then one ACT Sign / DVE is_ge
                over all banks."""
                lo_nt, w = win[(pair, mt)]
                t0 = D2_GROUP * k
                ntiles = min(D2_GROUP, w - t0)
                d2_ps = d2_pool.tile([P, D2_GROUP, MW], f32, tag="d2")
                for j in range(ntiles):
                    nt = lo_nt + t0 + j
                    g = quad_ctr[0] % 4
                    quad_ctr[0] += 1
                    nc.tensor.matmul(
                        d2_ps[:, j, :],
                        geoL_sb[32 * g : 32 * g + KGEO, pair, nt * P : (nt + 1) * P],
                        geoR_sb[32 * g : 32 * g + KGEO, pair, mt * MW : (mt + 1) * MW],
                        start=True,
                        stop=True,
                        tile_position=(32 * g, 0),
                    )
                col = sgn_col[(pair, mt)] + k
                src = d2_ps[:, 0:ntiles, :]
                dst = mask_t[:, t0 : t0 + ntiles, :]
                if _step_engine(pair, mt) == "act":
                    nc.scalar.activation(
                        dst,
                        src,
                        mybir.ActivationFunctionType.Sign,
                        accum_out=sgn_acc[:, col : col + 1],
                    )
                else:
                    nc.vector.tensor_scalar(
                        out=dst,
                        in0=src,
                        scalar1=0.0,
                        scalar2=0.0,
                        op0=mybir.AluOpType.is_ge,
                        op1=mybir.AluOpType.add,
                        accum_out=sgn_acc[:, col : col + 1],
                    )

            def emit_g_unit(pair, mt, mask_t, dc, g_ps):
                """G^T[d, m] = sum_{n in window} fhat_j[n, d] * mask[n, m],
                then fused extraction into ext_acc."""
                lo_nt, w = win[(pair, mt)]
                npairs = w // 2
                for t in range(npairs):
                    a0 = lo_nt + 2 * t
                    nc.tensor.matmul(
                        g_ps[:],
                        fj_sb[:, pair, a0 : a0 + 2, dc * P : (dc + 1) * P],
                        mask_t[:, 2 * t : 2 * t + 2, :],
                        start=(t == 0),
                        stop=(t == npairs - 1 and w % 2 == 0),
                        perf_mode=mybir.MatmulPerfMode.DoubleRow,
                    )
                if w % 2 == 1:
                    a0 = lo_nt + w - 1
                    nc.tensor.matmul(
                        g_ps[:],
                        fj_sb[:, pair, a0, dc * P : (dc + 1) * P],
                        mask_t[:, w - 1, :],
                        start=(w == 1),
                        stop=True,
                    )
                ecol = (pair * MT + mt) * DC + dc
                scr = scratch_pool.tile([P, MW], f32, tag="scr")
                nc.vector.scalar_tensor_tensor(
                    out=scr[:],
                    in0=g_ps[:],
                    scalar=0.0,
                    in1=fi_sb[:, pair, dc, mt * MW : (mt + 1) * MW],
                    op0=mybir.AluOpType.add,
                    op1=mybir.AluOpType.mult,
                    accum_out=ext_acc[:, ecol : ecol + 1],
                )

            def g_units(pair, mt, mask_t):
                units = []
                if win[(pair, mt)][1] == 0:
                    return units
                for dc in range(DC):
                    g_ps = g_pool.tile([P, MW], f32, tag="g")
                    units.append(
                        lambda p=pair, m=mt, k=mask_t, d=dc, g=g_ps:
                            emit_g_unit(p, m, k, d, g)
                    )
                return units

            def emit_body():
                if PIPE:
                    # Virtual-clock scheduler: two work streams (ACT mask
                    # ops; DVE mask ops + G-unit extractions) are emitted in
                    # estimated-engine-time order so both engines stay fed.
                    # G units of a step become eligible once all its mask
                    # ops are emitted.
                    from collections import deque

                    def op_cost(eng, ntiles):
                        if eng == "act":
                            return (ntiles * MW + 352) / 1.2 + 187
                        return (ntiles * MW + 120) / 0.96

                    act_items = deque()
                    dve_items = deque()
                    for k in range(N_STEPS):
                        s = steps[k]
                        eng = _step_engine(*s)
                        m = mask_pool.tile([P, wmax, MW], fp8, tag="mask")
                        items = act_items if eng == "act" else dve_items
                        w = win[s][1]
                        for q in range(nops[s]):
                            ntiles = min(D2_GROUP, w - D2_GROUP * q)
                            last = q == nops[s] - 1
                            items.append(
                                (
                                    op_cost(eng, ntiles),
                                    lambda s=s, q=q, m=m, last=last: (
                                        emit_d2_op(s[0], s[1], q, m),
                                        last and gq.extend(g_units(s[0], s[1], m)),
                                    ),
                                )
                            )
                    gq = deque()  # eligible G units -> DVE stream
                    t_act = 0.0
                    t_dve = 0.0
                    while act_items or dve_items or gq:
                        dve_ready = dve_items or gq
                        if act_items and (not dve_ready or t_act <= t_dve):
                            cost, fn = act_items.popleft()
                            fn()
                            t_act += cost
                        elif dve_items and (not gq or len(gq) < 2):
                            cost, fn = dve_items.popleft()
                            fn()
                            t_dve += cost
                        else:
                            absorb_feat_tick()
                            gq.popleft()()
                            t_dve += 658.0
                else:
                    gq = deque()
                    for s in steps:
                        m = mask_pool.tile([P, wmax, MW], fp8, tag="mask")
                        for k in range(nops[s]):
                            emit_d2_op(s[0], s[1], k, m)
                        absorb_feat_tick()
                        for u in g_units(s[0], s[1], m):
                            u()

            if reps == 1:
                emit_body()
            else:
                nc.vector.memset(sgn_acc[:], 0.0)
                nc.vector.memset(ext_acc[:], 0.0)
                with tc.For_i(0, reps, 1):
                    emit_body()

            nc.sync.dma_start(out=out_d[:, 0:n_sgn], in_=sgn_acc[:])
            nc.sync.dma_start(out=out_d[:, n_sgn : n_sgn + n_ext], in_=ext_acc[:])

    if split_waits:
        _split_multi_waits(nc)
    return nc


def _split_multi_waits(nc):
    """Walrus rejects >1 sync-wait on compute/DMA instruction encodings.

    Hoist all but one wait of any multi-wait instruction onto standalone
    InstEventSemaphore instructions inserted immediately before it on the
    same engine queue (semantically identical: every wait must pass before
    the instruction dispatches either way).
    """
    import concourse.mybir as mybir

    n_split = 0
    for bb in nc.main_func.blocks:
        new_list = []
        for inst in bb.instructions:
            si = inst.sync_info
            if (
                si is not None
                and si.on_wait
                and len(si.on_wait) > 1
                and not isinstance(inst, mybir.InstEventSemaphore)
            ):
                waits = list(si.on_wait)
                for k, w in enumerate(waits[:-1]):
                    n_split += 1
                    new_list.append(
                        mybir.InstEventSemaphore(
                            name=f"{inst.name}-hw{k}",
                            engine=inst.engine,
                            ins=[],
                            outs=[],
                            sync_info=mybir.SyncInfo(on_wait=[w], on_update=[]),
                        )
                    )
                inst.sync_info = mybir.SyncInfo(
                    on_wait=[waits[-1]], on_update=list(si.on_update or [])
                )
            new_list.append(inst)
        bb.instructions[:] = new_list
    return n_split


def _split2(x):
    """fp64 -> (hi, lo) bf16 such that hi+lo ~ x to ~17 mantissa bits."""
    hi = x.astype(BF16)
    lo = (x - hi.astype(np.float64)).astype(BF16)
    return hi, lo


def _split3(x):
    hi = x.astype(BF16)
    r = x - hi.astype(np.float64)
    mid = r.astype(BF16)
    lo = (r - mid.astype(np.float64)).astype(BF16)
    return hi, mid, lo


def prepare(features, pts_src, pts_dst, height, width):
    """Sort points by x per pair, build per-core dense device inputs and
    the shared (union over cores) per-step window list.

    Returns (in_maps, windows) where windows is a tuple over the
    N_STEPS steps of (lo_nt, w)."""
    height = int(height)
    width = int(width)
    scale32 = np.array([(width - 1) * 0.5, (height - 1) * 0.5], dtype=np.float32)

    # Match the reference's fp32 denormalization rounding, then center.
    ps32 = (np.asarray(pts_src).astype(np.float32) + np.float32(1.0)) * scale32
    pd32 = (np.asarray(pts_dst).astype(np.float32) + np.float32(1.0)) * scale32
    psc = ps32.astype(np.float64) - scale32.astype(np.float64)   # [B,N,2]
    pdc = pd32.astype(np.float64) - scale32.astype(np.float64)   # [B,B,N,2]

    src_ord = np.argsort(psc[..., 0], axis=-1, kind="stable")    # [B, N]
    dst_ord = np.argsort(pdc[..., 0], axis=-1, kind="stable")    # [B, B, N]

    # Normalized features quantized exactly as the device sees them.
    f64 = np.asarray(features).astype(np.float64)
    norms = np.sqrt((f64 * f64).sum(-1, keepdims=True))
    fhat8 = (f64 / norms).astype(BF16).astype(FP8)               # [B, N, D]

    pair_list = [(p // B, p % B) for p in range(B * B)]

    in_maps = []
    ranges = []   # per core: per step (lo_idx, hi_idx) raw src index range
    for cidx in range(N_CORES):
        pairs = [2 * cidx, 2 * cidx + 1]
        geoL_core = np.zeros((P, PAIRS_PER_CORE, N), dtype=BF16)
        geoR_core = np.zeros((P, PAIRS_PER_CORE, N), dtype=BF16)
        fj_core = np.zeros((P, PAIRS_PER_CORE, NT, D), dtype=FP8)
        fi_core = np.zeros((P, PAIRS_PER_CORE, DC, N), dtype=FP8)
        rng_core = []
        for q, pr in enumerate(pairs):
            i_, j_ = pair_list[pr]
            so = src_ord[i_]
            do = dst_ord[i_, j_]
            pso = psc[i_][so]          # [N, 2] sorted src coords (centered)
            pdo = pdc[i_, j_][do]      # [N, 2] sorted dst coords
            fj_s = fhat8[j_][so]       # [N, D] src features, sorted
            fi_s = fhat8[i_][do]       # [N, D] dst features, sorted

            phx, plx = _split2(pso[:, 0])
            phy, ply = _split2(pso[:, 1])
            qhx, qlx = _split2(pdo[:, 0])
            qhy, qly = _split2(pdo[:, 1])
            sh, sm, sl = _split3(
                RADIUS2
                - (
                    (phx.astype(np.float64) + plx.astype(np.float64)) ** 2
                    + (phy.astype(np.float64) + ply.astype(np.float64)) ** 2
                )
            )
            tq = (
                (qhx.astype(np.float64) + qlx.astype(np.float64)) ** 2
                + (qhy.astype(np.float64) + qly.astype(np.float64)) ** 2
            )
            th, tm, tl = _split3(tq)
            ones = np.ones(N, dtype=BF16)
            p2hx = (2.0 * phx.astype(np.float64)).astype(BF16)
            p2lx = (2.0 * plx.astype(np.float64)).astype(BF16)
            p2hy = (2.0 * phy.astype(np.float64)).astype(BF16)
            p2ly = (2.0 * ply.astype(np.float64)).astype(BF16)
            gl = np.stack(
                [p2hx, p2hx, p2lx, p2lx, p2hy, p2hy, p2ly, p2ly,
                 sh, sm, sl, -ones, -ones, -ones], axis=0)   # [14, N]
            gr = np.stack(
                [qhx, qlx, qhx, qlx, qhy, qly, qhy, qly,
                 ones, ones, ones, th, tm, tl], axis=0)      # [14, N]
            for g in range(4):
                geoL_core[32 * g : 32 * g + KGEO, q] = gl
                geoR_core[32 * g : 32 * g + KGEO, q] = gr

            fj_core[:, q] = fj_s.reshape(NT, P, D).transpose(1, 0, 2)
            fi_core[:, q] = fi_s.T.reshape(DC, P, N).transpose(1, 0, 2)

            xsrc = pso[:, 0]
            xdst = pdo[:, 0]
            for mt in range(MT):
                xlo = xdst[mt * MW] - MARGIN
                xhi = xdst[(mt + 1) * MW - 1] + MARGIN
                lo = int(np.searchsorted(xsrc, xlo, side="left"))
                hi = int(np.searchsorted(xsrc, xhi, side="right"))
                rng_core.append((lo, hi))
        in_maps.append(
            {"geoL": geoL_core, "geoR": geoR_core, "fj": fj_core, "fiT": fi_core}
        )
        ranges.append(rng_core)

    # Shared (SPMD) windows: per step, the tile-aligned union over cores.
    windows = []
    for k in range(N_STEPS):
        lo = min(ranges[c][k][0] for c in range(N_CORES))
        hi = max(ranges[c][k][1] for c in range(N_CORES))
        lo_nt = lo // P
        hi_nt = -(-hi // P)
        windows.append((lo_nt, max(hi_nt - lo_nt, 1)))
    return in_maps, tuple(windows)


def _corrections(in_maps, windows):
    """Per core, per step, per dc: the +-1 affine correction term
    sum_{d in dc} (sum_{n in window} fj[n,d]) * (sum_{m in mtile} fi[d,m]),
    computed in fp64 over the exact fp8 values the device uses."""
    corr = []
    for cidx in range(N_CORES):
        fj_core = in_maps[cidx]["fj"].astype(np.float64)   # [P, pairs, NT, D]
        fi_core = in_maps[cidx]["fiT"].astype(np.float64)  # [P, pairs, DC, N]
        cc = []
        for k in range(N_STEPS):
            pair, mt = divmod(k, MT)
            lo_nt, w = windows[k]
            wsum = fj_core[:, pair, lo_nt : lo_nt + w].sum(axis=(0, 1))  # [D]
            msum = fi_core[:, pair, :, mt * MW : (mt + 1) * MW].sum(axis=2)  # [P, DC]
            cc.append(
                [float(np.dot(wsum[dc * P : (dc + 1) * P], msum[:, dc]))
                 for dc in range(DC)]
            )
        corr.append(cc)
    return corr


def _combine(results, windows, corr):
    """Host-side exact combination of per-core partial sums."""
    nops = [_nops(w) for (_, w) in windows]
    n_sgn = max(sum(nops), 1)
    a_total = 0.0
    b_total = 0.0
    for cidx, res in enumerate(results):
        out = res["out"].astype(np.float64)
        sgn = out[:, 0:n_sgn].sum(axis=0)     # per mask-op column
        ext = out[:, n_sgn:].sum(axis=0)      # per (pair, mt, dc) column
        c = 0
        for k in range(N_STEPS):
            pair, mt = divmod(k, MT)
            lo_nt, w = windows[k]
            eng = _step_engine(pair, mt)
            for op in range(nops[k]):
                ntiles = min(D2_GROUP, w - D2_GROUP * op)
                v = float(sgn[c])
                c += 1
                if eng == "act":
                    a_total += 0.5 * (v + P * MW * ntiles)
                else:
                    a_total += v
            for dc in range(DC):
                e = float(ext[(pair * MT + mt) * DC + dc])
                if eng == "act":
                    b_total += 0.5 * (e + corr[cidx][k][dc])
                else:
                    b_total += e
    return a_total, b_total


def kernel(features, pts_src, pts_dst, invis_idx, height, width):
    global LAST
    del invis_idx  # unused by the reference computation

    in_maps, windows = prepare(features, pts_src, pts_dst, height, width)
    corr = _corrections(in_maps, windows)

    from concourse.bass_utils import run_bass_kernel_spmd

    if windows not in _CACHE:
        _CACHE[windows] = _build_bass(windows)
    nc = _CACHE[windows]

    LAST = run_bass_kernel_spmd(nc, in_maps, core_ids=list(range(N_CORES)))

    a_total, b_total = _combine(LAST.results, windows, corr)
    loss = (a_total - b_total) / max(a_total, 1.0)
    return np.float32(loss)
